# revision 61
# baseline (speedup 1.0000x reference)
"""Linformer attention Trainium2 kernel.

Sharding: 8 cores = 4 batches x 2 head-groups (8 heads each).
Key fact: reference reshapes (B,H,N,d)->(B,N,C) WITHOUT head transpose, so
output row n_idx depends only on head h = n_idx//256.  Each core therefore
produces a fully independent slice final[b, 2048*g:(g+1)*2048, :] - no
collectives needed.

Per-core layout (everything transposed on host for contiguous DMA):
  xT      [C=1024, N=4096]      x[b].T
  wqkvT   [C=1024, M=1536]      Wqkv rows for this head group, transposed.
                                cols 0:512 = q (head-major, 64 each);
                                cols 512:1536 = k/v interleaved per head:
                                even h -> [k_h | v_h], odd h -> [v_h | k_h]
                                (so the k half always lands at partition
                                base 64*(h%2), matching where q_h lives).
  eT      [8, N=4096, K=256]    E[h].T per head
  wprojT  [C=1024, C=1024]      Wproj.T
  bproj   [1024, 1]
  outT    [1024, 2048]          final.T slice (host re-transposes)

Device phases (single NEFF, Tile framework, bf16 data / fp32 psum):
  1. qT = (wqkvT_q).T @ xT          [512, 4096]   (resident SBUF)
     kv[n,m] tiles = xT.T @ wqkvT_kv              (streamed, direct layout)
     klm/vlm:  kv(n,128-col-slice).T @ eT -> [128, 256] accum over n
  2. software-pipelined per head pair p (A interleaved with B of pair p-1
     so ACT's exp stream hides behind PE work; C after both B's):
     A: dotT[lm,n] = klmT.T @ qT_h ; exp via ACT(scale=1/8);
        rowsums[1,n] = ones128.T @ exp (PE partition-reduce, lagged one
        nt step behind the dots) -> ri row holds raw sums (bf16)
     B: y: strided rhs (exp[:, j::16]) gives the no-transpose reshape;
        even j accumulates into psum rows 0:64, odd j rows 64:128, two
        j-pairs share a [128,512] psum tile; sum-broadcast via ones1
        matmul into a second [128,512] psum; reciprocal_approx_fast on
        the broadcast (128-wide DVE, doubles as the PSUM->SBUF stage);
        one paired DVE multiply emits normalized ort block.  No DMA
        round trips anywhere in the softmax path.
  3. C: finalT = wprojT.T @ orts + bias (head-pair free-512 rhs); DMA out
"""

import sys

sys.path.insert(0, "/opt/trn_rl_repo")

import numpy as np
import ml_dtypes
from contextlib import ExitStack

import concourse.bass as bass
import concourse.tile as tile
from concourse import bacc
from concourse import mybir
from concourse.bass_utils import run_bass_kernel_spmd
from concourse.masks import make_identity

B, N, C = 4, 4096, 1024
H, K_LM = 16, 256
D = C // H  # 64
HPC = 8  # heads per core
F32 = mybir.dt.float32
BF16 = mybir.dt.bfloat16
FE = mybir.ActivationFunctionType

N_BLK = 1024
N_BLKS = N // N_BLK  # 4
N_SUB = 128
SUBS = N_BLK // N_SUB  # 8


def build_program(rep=1):
    """rep>1 repeats the whole computation in one NEFF (for wall-clock
    device-time calibration: marginal(R)-marginal(1) = (R-1)*exec)."""
    nc = bacc.Bacc("TRN2", target_bir_lowering=False, debug=False, num_devices=8)

    xT = nc.dram_tensor("xT", [C, N], BF16, kind="ExternalInput").ap()
    wqkvT = nc.dram_tensor("wqkvT", [C, 3 * D * HPC], BF16, kind="ExternalInput").ap()
    eT = nc.dram_tensor("eT", [HPC, N, K_LM], BF16, kind="ExternalInput").ap()
    wprojT = nc.dram_tensor("wprojT", [C, C], BF16, kind="ExternalInput").ap()
    bproj = nc.dram_tensor("bproj", [C, 1], F32, kind="ExternalInput").ap()
    outT = nc.dram_tensor("outT", [C, HPC * 256], F32, kind="ExternalOutput").ap()

    with tile.TileContext(nc) as tc:
        for r in range(rep):
            _emit_body(nc, tc, xT, wqkvT, eT, wprojT, bproj, outT, sfx=f"_{r}")

    nc.compile()
    return nc


def _emit_body(nc, tc, xT, wqkvT, eT, wprojT, bproj, outT, sfx=""):
    with ExitStack() as ctx:
        singles = ctx.enter_context(tc.tile_pool(name=f"singles{sfx}", bufs=1))
        qres = ctx.enter_context(tc.tile_pool(name=f"qres{sfx}", bufs=1))

        ident = singles.tile([128, 128], BF16)
        make_identity(nc, ident)

        # bias_sb[p, ct] = bproj[ct*128 + p]
        bias_sb = singles.tile([128, 8], F32)
        nc.sync.dma_start(out=bias_sb, in_=bproj.rearrange("(a b) o -> b (a o)", b=128))

        ones128 = singles.tile([128, 1], BF16)
        nc.vector.memset(ones128, 1.0)
        ones1 = singles.tile([1, 64], BF16)
        nc.vector.memset(ones1, 1.0)

        # dummy exp warms the ACT function table during phase 1 so the
        # 1.3us table load isn't on the phase-2 critical path
        actwarm = singles.tile([1, 1], F32)
        nc.scalar.activation(actwarm, ones1[0:1, 0:1], FE.Exp)

        qT = [qres.tile([128, N], BF16, tag=f"qT{i}", name=f"qT{i}{sfx}") for i in range(4)]
        # landmark tiles, bf16: even h rows 0:64 k | 64:128 v; odd h flipped
        klmb = qres.tile([128, HPC, K_LM], BF16)

        # ---------------- Phase 1: qkv + landmark projection ----------------
        with tc.tile_pool(name=f"wqp{sfx}", bufs=1) as wqp, \
             tc.tile_pool(name=f"xtp{sfx}", bufs=16) as xtp, \
             tc.tile_pool(name=f"ep{sfx}", bufs=8) as ep, \
             tc.tile_pool(name=f"kvp{sfx}", bufs=18) as kvp, \
             tc.tile_pool(name=f"klmp{sfx}", bufs=1) as klmp, \
             tc.tile_pool(name=f"ps1{sfx}", bufs=2, space="PSUM") as ps1, \
             tc.tile_pool(name=f"ps_kv{sfx}", bufs=3, space="PSUM") as ps_kv, \
             tc.tile_pool(name=f"ps_lm{sfx}", bufs=2, space="PSUM") as ps_lm:

            klm = klmp.tile([128, HPC, K_LM], F32)

            # interleave weight loads with the first x block (in column
            # halves) so the first q matmuls start after ~1us of DMA;
            # kv weights and second halves stream in behind
            w_sb = []
            xts0 = []
            for ct in range(8):
                t = wqp.tile([128, 3 * D * HPC], BF16, tag=f"wqkv{ct}")
                nc.sync.dma_start(
                    out=t[:, 0:512], in_=wqkvT[ct * 128 : (ct + 1) * 128, 0:512]
                )
                w_sb.append(t)
                xt0 = xtp.tile([128, N_BLK], BF16, tag="xt")
                nc.sync.dma_start(
                    out=xt0[:, 0:512], in_=xT[ct * 128 : (ct + 1) * 128, 0:512]
                )
                xts0.append(xt0)
            for ct in range(8):
                nc.sync.dma_start(
                    out=xts0[ct][:, 512:1024],
                    in_=xT[ct * 128 : (ct + 1) * 128, 512:1024],
                )
            for ct in range(8):
                nc.sync.dma_start(
                    out=w_sb[ct][:, 512:1536],
                    in_=wqkvT[ct * 128 : (ct + 1) * 128, 512:1536],
                )

            for nb in range(N_BLKS):
                nsl = bass.ts(nb, N_BLK)
                if nb == 0:
                    xts = xts0
                else:
                    xts = []
                    for ct in range(8):
                        t = xtp.tile([128, N_BLK], BF16, tag="xt")
                        nc.sync.dma_start(out=t, in_=xT[ct * 128 : (ct + 1) * 128, nsl])
                        xts.append(t)
                # q: out[m(128), n(512)] ; lhsT = wqkvT[:, mt*128:...]
                # nh outer so nb0's first column-half is consumed as it lands
                for nh in range(2):
                    for mt in range(4):
                        pq = ps1.tile([128, 512], F32, tag="pq")
                        for ct in range(8):
                            nc.tensor.matmul(
                                pq,
                                w_sb[ct][:, mt * 128 : (mt + 1) * 128],
                                xts[ct][:, nh * 512 : (nh + 1) * 512],
                                start=(ct == 0),
                                stop=(ct == 7),
                            )
                        nc.vector.tensor_copy(
                            qT[mt][:, nb * N_BLK + nh * 512 : nb * N_BLK + (nh + 1) * 512],
                            pq,
                        )

                # eT loads issued after the q section so the first block's
                # q matmuls aren't starved behind eT transfers (HWDGE is
                # serial; SP-queue order = consumption order)
                ets = []
                for h in range(HPC):
                    et = ep.tile([128, SUBS, K_LM], BF16, tag="et")
                    nc.sync.dma_start(
                        out=et,
                        in_=eT[h, nsl, :].rearrange("(s p) k -> p s k", p=N_SUB),
                    )
                    ets.append(et)

                # kv: out[n(128), m(1024 interleaved)] ; lhsT = xT col slice
                kvs = []
                for s in range(SUBS):
                    kvt = kvp.tile([128, 2 * D * HPC], BF16, tag="kv")
                    for half in range(2):
                        pkv = ps_kv.tile([128, 512], F32, tag="pkv")
                        msl = bass.ds(512 + half * 512, 512)
                        for ct in range(8):
                            nc.tensor.matmul(
                                pkv,
                                xts[ct][:, s * 128 : (s + 1) * 128],
                                w_sb[ct][:, msl],
                                start=(ct == 0),
                                stop=(ct == 7),
                            )
                        nc.vector.tensor_copy(
                            kvt[:, half * 512 : (half + 1) * 512], pkv
                        )
                    kvs.append(kvt)

                # landmark accumulation per head
                for h in range(HPC):
                    plm = ps_lm.tile([128, K_LM], F32, tag="plm")
                    for s in range(SUBS):
                        nc.tensor.matmul(
                            plm,
                            kvs[s][:, h * 128 : (h + 1) * 128],
                            ets[h][:, s, :],
                            start=(s == 0),
                            stop=(s == SUBS - 1),
                        )
                    if nb == 0:
                        nc.vector.tensor_copy(klm[:, h, :], plm)
                    elif nb < N_BLKS - 1:
                        nc.vector.tensor_add(klm[:, h, :], klm[:, h, :], plm)
                    else:
                        # final add goes straight to the bf16 copy phase 2 uses
                        nc.vector.tensor_add(klmb[:, h, :], klm[:, h, :], plm)

        # ---------------- Phase 2+3: attention + projection per head --------
        with tc.tile_pool(name=f"wpp{sfx}", bufs=1) as wpp, \
             tc.tile_pool(name=f"expp{sfx}", bufs=4) as expp, \
             tc.tile_pool(name=f"vop{sfx}", bufs=8) as vop, \
             tc.tile_pool(name=f"rip{sfx}", bufs=4) as rip, \
             tc.tile_pool(name=f"orp{sfx}", bufs=2) as orp, \
             tc.tile_pool(name=f"fout{sfx}", bufs=4) as fout, \
             tc.tile_pool(name=f"ps_dot{sfx}", bufs=2, space="PSUM") as ps_dot, \
             tc.tile_pool(name=f"ps_s{sfx}", bufs=2, space="PSUM") as ps_s, \
             tc.tile_pool(name=f"ps_yr{sfx}", bufs=2, space="PSUM") as ps_yr, \
             tc.tile_pool(name=f"ps_f{sfx}", bufs=2, space="PSUM") as ps_f:

            wp_sb = []  # wprojT tiles [128, 1024] per ci-tile
            for ct in range(8):
                t = wpp.tile([128, C], BF16, tag=f"wproj{ct}")
                nc.sync.dma_start(out=t, in_=wprojT[ct * 128 : (ct + 1) * 128, :])
                wp_sb.append(t)

            # software pipeline: A(pair)=dot/exp/rowsum, B(h)=y/bc/normalize,
            # C(pair)=projection over head pairs (free-512 rhs).  A's 8
            # nt-steps interleave with B's 8 jpp-steps so ACT's exp stream
            # (2.3us/step) hides behind ~3.8us of PE work per step.
            def stageA_alloc(p):
                sts = []
                for h in (2 * p, 2 * p + 1):
                    pk = 64 * (h % 2)  # partition base of q_h and k_h
                    pv = 64 - pk       # partition base of v_h
                    vlmT = []
                    for half in range(2):
                        vt = vop.tile([128, 64], BF16, tag="vlmT")
                        pt = ps_dot.tile([128, 64], BF16, tag="pd")
                        vsrc = klmb[pv : pv + 64, h, half * 128 : (half + 1) * 128]
                        idn = ident[pv : pv + 64, pv : pv + 64]
                        nc.tensor.transpose(pt, vsrc, idn)
                        nc.vector.tensor_copy(vt, pt)
                        vlmT.append(vt)
                    exps = [
                        expp.tile(
                            [128, N], BF16, tag=f"exp{half}", name=f"ex{h}_{half}{sfx}"
                        )
                        for half in range(2)
                    ]
                    ri = rip.tile([1, N], BF16, tag="ri", name=f"ri{h}{sfx}")
                    sts.append((exps, ri, vlmT))
                return sts

            def _dots(p, sts, hh, nt):
                h = 2 * p + hh
                pk = 64 * (h % 2)
                qh = qT[h // 2][pk : pk + 64, :]
                for half in range(2):
                    klmh = klmb[pk : pk + 64, h, half * 128 : (half + 1) * 128]
                    pd = ps_dot.tile([128, 512], F32, tag="pd")
                    nc.tensor.matmul(
                        pd, klmh, qh[:, bass.ts(nt, 512)], start=True, stop=True
                    )
                    nc.scalar.activation(
                        sts[hh][0][half][:, bass.ts(nt, 512)],
                        pd, FE.Exp, scale=0.125,
                    )

            def _rowsum(sts, hh, nt):
                exps, ri, _ = sts[hh]
                pss = ps_s.tile([1, 512], F32, tag="pss")
                nc.tensor.matmul(
                    pss, ones128, exps[0][:, bass.ts(nt, 512)],
                    start=True, stop=False,
                )
                nc.tensor.matmul(
                    pss, ones128, exps[1][:, bass.ts(nt, 512)],
                    start=False, stop=True,
                )
                # ri holds the raw row SUMS (bf16); reciprocal happens
                # later on the 128-partition broadcast (full DVE width)
                nc.vector.tensor_copy(ri[:, bass.ts(nt, 512)], pss)

            def stageA_step(p, sts, nt):
                _dots(p, sts, 0, nt)
                _dots(p, sts, 1, nt)
                if nt > 0:
                    _rowsum(sts, 0, nt - 1)
                    _rowsum(sts, 1, nt - 1)

            def stageA_tail(sts):
                _rowsum(sts, 0, 7)
                _rowsum(sts, 1, 7)

            def stageB_step(h, st, orts2, jpp):
                exps, ri, vlmT = st
                he = h % 2
                if True:  # jp-pair: jp = 2*jpp + jpi
                    pyy = ps_yr.tile([128, 512], F32, tag="pyy")
                    pr2 = ps_f.tile([128, 512], F32, tag="pf")
                    for jpi in range(2):
                        jp = 2 * jpp + jpi
                        csl = bass.ds(jpi * 256, 256)
                        for jo in range(2):
                            j = 2 * jp + jo
                            psl = bass.ds(jo * 64, 64)
                            for half in range(2):
                                rhs = exps[half].rearrange(
                                    "p (q j) -> p j q", j=16
                                )[:, j, :]
                                nc.tensor.matmul(
                                    pyy[psl, csl],
                                    vlmT[half],
                                    rhs,
                                    start=(half == 0),
                                    stop=(half == 1),
                                )
                            rrhs = ri.rearrange("o (q j) -> o j q", j=16)[:, j, :]
                            nc.tensor.matmul(
                                pr2[psl, csl], ones1, rrhs, start=True, stop=True
                            )
                    # reciprocal of the broadcast sums (128-wide DVE) doubles
                    # as the PSUM->SBUF stage, then one paired multiply
                    prf = vop.tile([128, 512], F32, tag="prf")
                    nc.vector.reciprocal_approx_fast(out=prf, in_=pr2)
                    oview = orts2.rearrange("p a (t q) -> p a t q", t=2)[
                        :, 2 * jpp : 2 * jpp + 2, he, :
                    ]
                    pview = pyy.rearrange("p (a q) -> p a q", a=2)
                    bview = prf.rearrange("p (a q) -> p a q", a=2)
                    nc.vector.tensor_mul(oview, pview, bview)

            def stageC(p, orts2):
                for co in range(8):
                    pf = ps_f.tile([128, 512], F32, tag="pf")
                    for ci in range(8):
                        nc.tensor.matmul(
                            pf,
                            wp_sb[ci][:, co * 128 : (co + 1) * 128],
                            orts2[:, ci, :],
                            start=(ci == 0),
                            stop=(ci == 7),
                        )
                    fo = fout.tile([128, 512], F32, tag="fo")
                    nc.scalar.activation(
                        fo, pf, FE.Identity, bias=bias_sb[:, co : co + 1]
                    )
                    nc.sync.dma_start(
                        out=outT[
                            co * 128 : (co + 1) * 128, p * 512 : (p + 1) * 512
                        ],
                        in_=fo,
                    )

            # warmup pair 0, then per iteration interleave A(p+1) nt-steps
            # with B(2p)/B(2p+1) jpp-steps, finishing with C(p)
            sts = stageA_alloc(0)
            for k in range(8):
                stageA_step(0, sts, k)
            stageA_tail(sts)
            for p in range(HPC // 2):
                orts2 = orp.tile([128, 8, 512], BF16, tag="ort", name=f"ort{p}{sfx}")
                nxt = stageA_alloc(p + 1) if p + 1 < HPC // 2 else None
                for k in range(8):
                    if nxt is not None:
                        stageA_step(p + 1, nxt, k)
                    stageB_step(2 * p + (k >= 4), sts[k >= 4], orts2, k % 4)
                if nxt is not None:
                    stageA_tail(nxt)
                sts = nxt
                stageC(p, orts2)


_NC_CACHE = None


def make_in_maps(x, Wqkv, E, Wproj, bproj):
    wprojT = np.ascontiguousarray(Wproj.T).astype(ml_dtypes.bfloat16)
    bp = np.ascontiguousarray(bproj.reshape(C, 1))

    in_maps = []
    for cid in range(8):
        b, g = cid // 2, cid % 2
        base = g * HPC * D
        q_rows = np.arange(base, base + HPC * D)
        kv_rows = []
        for h in range(HPC):
            k_rows = np.arange(C + base + h * D, C + base + (h + 1) * D)
            v_rows = np.arange(2 * C + base + h * D, 2 * C + base + (h + 1) * D)
            kv_rows.extend([k_rows, v_rows] if h % 2 == 0 else [v_rows, k_rows])
        rows = np.concatenate([q_rows] + kv_rows)
        in_maps.append(
            {
                "xT": np.ascontiguousarray(x[b].T).astype(ml_dtypes.bfloat16),
                "wqkvT": np.ascontiguousarray(Wqkv[rows].T).astype(ml_dtypes.bfloat16),
                "eT": np.ascontiguousarray(
                    E[g * HPC : (g + 1) * HPC].transpose(0, 2, 1)
                ).astype(ml_dtypes.bfloat16),
                "wprojT": wprojT,
                "bproj": bp,
            }
        )
    return in_maps


def assemble_output(results):
    out = np.empty((B, N, C), dtype=np.float32)
    for cid in range(8):
        b, g = cid // 2, cid % 2
        out[b, g * 2048 : (g + 1) * 2048, :] = results[cid]["outT"].T
    return out


def kernel(x, Wqkv, E, Wproj, bproj, **_):
    global _NC_CACHE
    x = np.asarray(x, dtype=np.float32)
    Wqkv = np.asarray(Wqkv, dtype=np.float32)
    E = np.asarray(E, dtype=np.float32)
    Wproj = np.asarray(Wproj, dtype=np.float32)
    bproj = np.asarray(bproj, dtype=np.float32)

    if _NC_CACHE is None:
        _NC_CACHE = build_program()
    nc = _NC_CACHE

    in_maps = make_in_maps(x, Wqkv, E, Wproj, bproj)
    res = run_bass_kernel_spmd(nc, in_maps, core_ids=list(range(8)))
    return assemble_output(res.results)
